# revision 29
# baseline (speedup 1.0000x reference)
"""ColorConstancy (multi-scale retinex) Trainium2 kernel.

Full-input contract: kernel(**inputs) takes the unsharded inputs from
setup_inputs() and returns the full (16, 3, 512, 512) float32 output.

Strategy (pure data parallel, batch sharded across 8 cores; 6 planes/core):
  log_img = ln(x + 1e-8)
  illum   = sum_s w_s * gauss2d_s(log_img)        (sigmas 2, 4, 8)
  refl    = log_img - illum
  out     = clip(exp((refl - mean) / (std_ddof1 + 1e-8)), 0, 1)

The 2-D Gaussian is separable: gauss2d_s(X) = U_s @ X @ U_s with U_s the
banded symmetric Toeplitz matrix of the 1-D kernel. Each pass is computed on
the TensorEngine as  pass(D) = D^T @ V  (lhsT = D blocks, rhs = V blocks), so
two passes give V^T X V = V X V with no explicit transposes. Folding
sqrt(w_s) into V_s makes illum a single PSUM accumulation in pass 2.

Key engine trick: pass 2's PSUM group also accumulates (-I)^T @ L, so PSUM
ends up holding  q = illum - log  = -refl  directly — the expensive DVE
tensor-tensor subtract (PSUM-source TT measured ~1.7us/tile) disappears.
Downstream works in q-space: bn_stats reads PSUM, clip(exp(z),0,1) =
exp(min(z,0)) becomes g = max(q, mean_q) (one DVE tensor-scalar from PSUM),
and y = exp(-rs*g + mean_q*rs) on ScalarE. PSUM evacuations are biased to
ACT (633ns/tile measured vs 1008 on DVE). Matmul operands are fp16
(full-rate PE, ~5e-4 rel precision).
"""

import numpy as np

N_CORES = 8
NPLANES = 6          # 2 batch images x 3 channels per core
H = W = 512
P = 128
NB = H // P          # 4 row blocks
CS = (6, 12, 24)     # band half-widths for sigma 2, 4, 8 (K = 13, 25, 49)
EPS = 1e-8
NPIX = H * W

_PROGRAM_CACHE = {}

# engine-balance knobs (tuned on HW):
#   EVAC_DVE_MOD: evacuations with (index % 12) < EVAC_DVE_MOD go to DVE
#   IDENT_TRICK: accumulate (-I)^T L into pass-2 PSUM (kills the DVE sub)
EVAC_DVE_MOD = 6
IDENT_TRICK = True
# LDW_SHARE: pass-1 kb-outer order + ldweights=False on the 2nd/3rd matmul
# sharing one stationary l16 block (saves ~1/3 of pass-1 LDWEIGHTS).
LDW_SHARE = False
# CLIP_ACT_N: how many of the 4 per-mb clip ops run on ACT as
# relu(q - mean_q) instead of on DVE as max(q - mean_q, 0); both produce
# r >= 0 and y = exp(-rs * r). Balances the clip load between engines.
CLIP_ACT_N = 0
# SEAM_SPLIT: halve the first plane's input DMA + Ln (along W) and the last
# plane's exp + output DMA, so the loop-seam critical path (barrier -> full
# DMA -> full Ln -> first matmul, and last exp -> full DMA -> barrier)
# pipelines at the halves. Steady-state planes stay whole-tile.
SEAM_SPLIT = True
# PIPE_DEPTH: software-pipeline lookahead in planes. 1 = pass 2 of plane p-1
# overlaps pass 1 of plane p; 2 = pass 2 trails by two planes, giving every
# engine a full extra plane of independent work (needs bufs=PIPE_DEPTH+1 on
# the front-side SBUF pools).
PIPE_DEPTH = 2
SBUF_BUFS = 2
PSA_BUFS = 3
PSI_BUFS = 4
# BACK_STYLE "max_exp": g = max(q, mq) on DVE (PSUM->SBUF), one whole-plane
#   exp on ACT, one whole-plane DMA out.
# BACK_STYLE "exp_min": per-mb exp straight from PSUM on ACT, per-mb clip-min
#   on DVE, per-mb DMA out (shorter serial tail, earlier PSUM release).
BACK_STYLE = "max_exp"
# QEVAC: copy q from PSUM to SBUF fp16 right away (engine "act" or "dve"),
# then bn_stats/max run on cheap SBUF fp16; psi pool shrinks to 2 and psA
# grows to 5. None = bn/max read PSUM directly (psi 4, psA 3).
QEVAC = None
# PE_KEEPALIVE: tiny dependency-chained matmuls through the first plane's
# input-DMA window and the last plane's stats/exp/DMA tail, so the PE never
# idles a full HAM activity window (idle > ~3.4us re-throttles the PE clock
# to 1.2 GHz; the For_i rep barrier otherwise exposes exactly such a gap).
PE_KEEPALIVE = True
# UNROLL: bodies per For_i iteration in the timing build (the For_i rep
# barrier + drain is a measurement-loop artifact; unrolling lets the
# software pipeline flow across plane-sets like the real one-shot kernel).
UNROLL = 2
# SIM_COMPAT: make pass-1 kb==0 matmuls write the full PSUM bank so CoreSim's
# chunk-granular pending-zero model accepts the banded accumulation (HW's
# per-element has_written handles the mixed case fine; sim-only, costs extra
# PE streaming so only enable for simulation analysis).
SIM_COMPAT = False


def _ncol(kb, c):
    """Output column range that input row block kb touches through a band-c kernel."""
    return max(0, P * kb - c), min(W, P * (kb + 1) + c)


def build_program(reps=1, ablate=()):
    """Build + compile the per-core Bass program. reps>1 wraps the whole
    computation in a hardware loop (for timing by subtraction).
    ablate: dev-only set of stage names to skip ("pe", "act", "evac") —
    output becomes wrong; used to attribute HW time."""
    ablate = set(ablate)
    import concourse.bacc as bacc
    import concourse.tile as tile
    from concourse import mybir, bass_isa

    f32 = mybir.dt.float32
    f16 = mybir.dt.float16
    AF = mybir.ActivationFunctionType

    # Steer Ln/Exp into the one table set containing both, so a single
    # ACT_TABLE_LOAD serves the whole kernel.
    from concourse.hw_specs import get_activation_tables
    _tabs = get_activation_tables("gen3")
    for _name, _fset in _tabs.items():
        if _name != "natural_log_exp_and_others":
            _fset.discard(AF.Ln)
            _fset.discard(AF.Exp)

    nc = bacc.Bacc("TRN2", target_bir_lowering=False, debug=False,
                   num_devices=N_CORES)
    x = nc.declare_dram_parameter("x", [NPLANES, H, W], f32, isOutput=False)
    vs = [nc.declare_dram_parameter(f"v{s}", [H, W], f16, isOutput=False)
          for s in range(3)]
    identd = nc.declare_dram_parameter("ident", [P, P], f16, isOutput=False)
    y = nc.declare_dram_parameter("y", [NPLANES, H, W], f32, isOutput=True)

    with tile.TileContext(nc) as tc:
        with (
            tc.tile_pool(name="consts", bufs=1) as consts,
            tc.tile_pool(name="xin", bufs=PIPE_DEPTH + 1) as xpool,
            tc.tile_pool(name="log16", bufs=PIPE_DEPTH + 1) as l16pool,
            tc.tile_pool(name="apool", bufs=PIPE_DEPTH + 1) as apool,
            tc.tile_pool(name="gpool", bufs=SBUF_BUFS) as gpool,
            tc.tile_pool(name="yout",
                         bufs=(6 if BACK_STYLE == "exp_min"
                               else SBUF_BUFS)) as ypool,
            tc.tile_pool(name="small", bufs=2) as spool,
            tc.tile_pool(name="psA", bufs=(5 if QEVAC else PSA_BUFS),
                         space="PSUM") as psA,
            tc.tile_pool(name="psI", bufs=(2 if QEVAC else PSI_BUFS),
                         space="PSUM") as psIp,
            tc.tile_pool(name="psS", bufs=1, space="PSUM") as psSp,
        ):
            # Banded blur matrices, resident for the whole kernel.
            # Layout [p, kb, n]: matrix row = kb*128 + p.
            V16 = []
            for s in range(3):
                vt = consts.tile([P, NB, W], f16, tag=f"v{s}")
                nc.sync.dma_start(
                    out=vt, in_=vs[s].rearrange("(kb p) n -> p kb n", p=P))
                V16.append(vt)
            epst = consts.tile([P, 1], f32, tag="eps")
            nc.vector.memset(epst, EPS)
            ones16 = consts.tile([P, P], f16, tag="ones16")
            nc.vector.memset(ones16, 1.0 / P)
            negI = consts.tile([P, P], f16, tag="negI")
            nc.sync.dma_start(out=negI, in_=identd[0:P])

            def emit_planes():
                state = {}

                def keepalive(rhs16):
                    # tiny matmul whose rhs depends on a tail/startup artifact
                    # — wakes the PE mid-gap so HAM never sees a full idle
                    # window. Output goes to the psS-tagged bank (rotated).
                    dps = psSp.tile([P, 2], f32, tag="psS", name="ka")
                    nc.tensor.matmul(dps[0:2, 0:2], ones16[:, 0:2], rhs16,
                                     start=True, stop=True)

                def front(p):
                    # load -> ln(fp16 out) -> pass 1 (A_s = L^T V_s)
                    xt = xpool.tile([P, NB, W], f32, tag="x")
                    xdram = x[p].rearrange("(kb q) w -> q kb w", q=P)
                    l16 = l16pool.tile([P, NB, W], f16, tag="l16")
                    halves = ((0, W // 2), (W // 2, W)) \
                        if (SEAM_SPLIT and p == 0) else ((0, W),)
                    for (lo, hi) in halves:
                        nc.sync.dma_start(out=xt[:, :, lo:hi],
                                          in_=xdram[:, :, lo:hi])
                        if PE_KEEPALIVE and p == 0 and lo == 0 \
                                and "pe" not in ablate:
                            from concourse import mybir as _mb
                            keepalive(xt.bitcast(_mb.dt.float16)[:, 0, 0:2])
                        if "act" in ablate:
                            nc.scalar.copy(out=l16[:, :, lo:hi],
                                           in_=xt[:, :, lo:hi])
                        else:
                            nc.scalar.activation(out=l16[:, :, lo:hi],
                                                 in_=xt[:, :, lo:hi],
                                                 func=AF.Ln,
                                                 bias=epst, scale=1.0)

                    A16 = [apool.tile([P, NB, W], f16, tag=f"a{s}", name=f"a16_{s}")
                           for s in range(3)]
                    if "evac" in ablate:
                        for s in range(3):
                            nc.vector.memset(A16[s], 0.5)
                    nevac = 0
                    if "pe" in ablate:
                        dummy = l16pool.tile([P, W], f32, tag="dummy",
                                             name="dummy")
                        nc.vector.memset(dummy, 0.25)
                    for mb in range(NB):
                        if "pe" in ablate:
                            ps = [dummy, dummy, dummy]
                        else:
                            ps = [psA.tile([P, W], f32, tag="ps", name=f"ps{i}")
                                  for i in range(3)]
                            if LDW_SHARE:
                                # kb outer: the 3 sigma matmuls of one kb
                                # share the stationary l16 block; skip the
                                # redundant LDWEIGHTS on the 2nd and 3rd.
                                for kb in range(NB):
                                    for s in range(3):
                                        lo, hi = _ncol(kb, CS[s])
                                        if SIM_COMPAT and kb == 0:
                                            lo, hi = 0, W
                                        inst = nc.tensor.matmul(
                                            ps[s][:, lo:hi],
                                            l16[:, kb, P * mb:P * (mb + 1)],
                                            V16[s][:, kb, lo:hi],
                                            start=(kb == 0),
                                            stop=(kb == NB - 1),
                                        )
                                        if s > 0:
                                            inst.ldweights = False
                            else:
                                for s in range(3):
                                    for kb in range(NB):
                                        lo, hi = _ncol(kb, CS[s])
                                        if SIM_COMPAT and kb == 0:
                                            lo, hi = 0, W
                                        nc.tensor.matmul(
                                            ps[s][:, lo:hi],
                                            l16[:, kb, P * mb:P * (mb + 1)],
                                            V16[s][:, kb, lo:hi],
                                            start=(kb == 0),
                                            stop=(kb == NB - 1),
                                        )
                        for s in range(3):
                            # evacuate PSUM -> SBUF fp16, split DVE/ACT
                            if "evac" in ablate:
                                continue
                            if nevac % 12 >= EVAC_DVE_MOD:
                                nc.scalar.copy(out=A16[s][:, mb, :], in_=ps[s])
                            else:
                                nc.vector.tensor_copy(out=A16[s][:, mb, :],
                                                      in_=ps[s])
                            nevac += 1
                    state[p] = (l16, A16)

                def back(p):
                    # pass 2: q = illum - log accumulated in PSUM
                    # (banded A_s^T V_s matmuls + (-I)^T L), then stats and
                    # y = exp(min(refl - mean, 0)/std) all in q-space.
                    l16, A16 = state.pop(p)
                    st6 = spool.tile([P, NB, 6], f32, tag="st6")
                    psis = []
                    q16 = (gpool.tile([P, NB, W], f16, tag="q16", name="q16")
                           if QEVAC else None)
                    for mb in range(NB):
                        if "pe" in ablate:
                            psi = l16[:, mb, :]
                        else:
                            psi = psIp.tile([P, W], f32, tag="psi",
                                            name=f"psi{mb}")
                            nc.tensor.matmul(psi, negI, l16[:, mb, :],
                                             start=True, stop=False)
                            for s in range(3):
                                for kb in range(NB):
                                    lo, hi = _ncol(kb, CS[s])
                                    nc.tensor.matmul(
                                        psi[:, lo:hi],
                                        A16[s][:, kb, P * mb:P * (mb + 1)],
                                        V16[s][:, kb, lo:hi],
                                        start=False,
                                        stop=(s == 2 and kb == NB - 1),
                                    )
                        if QEVAC == "act":
                            nc.scalar.copy(out=q16[:, mb, :], in_=psi)
                        elif QEVAC == "dve":
                            nc.vector.tensor_copy(out=q16[:, mb, :], in_=psi)
                        if QEVAC:
                            nc.vector.bn_stats(out=st6[:, mb, :],
                                               in_=q16[:, mb, :])
                        else:
                            nc.vector.bn_stats(out=st6[:, mb, :], in_=psi)
                            psis.append(psi)

                    # plane-wide mean/var of q: per-partition bn stats, then a
                    # ones-weights matmul sums [mean_p, E[x^2]_p] across
                    # partitions AND broadcasts to all partitions.
                    mv = spool.tile([P, 2], f32, tag="mv")
                    nc.vector.bn_aggr(out=mv, in_=st6)
                    t2 = spool.tile([P, 2], f16, tag="t2")
                    nc.vector.tensor_mul(out=t2[:, 1:2], in0=mv[:, 0:1],
                                         in1=mv[:, 0:1])
                    nc.vector.tensor_add(out=t2[:, 1:2], in0=t2[:, 1:2],
                                         in1=mv[:, 1:2])
                    nc.vector.tensor_copy(out=t2[:, 0:1], in_=mv[:, 0:1])
                    psS = psSp.tile([P, 2], f32, tag="psS")
                    nc.tensor.matmul(psS, ones16, t2, start=True, stop=True)

                    fin = spool.tile([P, 6], f32, tag="fin")
                    mq = fin[:, 0:1]     # mean of q  (= -mean(refl))
                    tmp = fin[:, 1:2]    # var -> rs
                    rs = fin[:, 2:3]
                    rsn = fin[:, 3:4]    # -rs
                    mqn = fin[:, 4:5]    # -mq (ACT relu bias)
                    nc.vector.tensor_copy(out=mq, in_=psS[:, 0:1])
                    sq = spool.tile([P, 1], f32, tag="sq")
                    nc.vector.tensor_mul(out=sq, in0=mq, in1=mq)
                    nc.vector.tensor_sub(out=tmp, in0=psS[:, 1:2], in1=sq)
                    # rs = 1/std = exp(-0.5*ln(var * N/(N-1)))  (ddof=1; the
                    # +eps on std is 1e-8 vs std~O(1), folded away). Ln/Exp
                    # share one ACT table set.
                    nc.scalar.activation(out=tmp, in_=tmp, func=AF.Ln,
                                         scale=float(NPIX) / (NPIX - 1))
                    nc.scalar.activation(out=rs, in_=tmp, func=AF.Exp,
                                         scale=-0.5)
                    nc.vector.tensor_scalar_mul(out=rsn, in0=rs, scalar1=-1.0)
                    if CLIP_ACT_N > 0:
                        nc.vector.tensor_scalar_mul(out=mqn, in0=mq,
                                                    scalar1=-1.0)

                    if BACK_STYLE == "max_exp":
                        # r = max(q - mq, 0); then y = exp(-rs*r)
                        gt = gpool.tile([P, NB, W], f16, tag="g")
                        if QEVAC:
                            nc.vector.tensor_scalar(
                                out=gt, in0=q16, scalar1=mq, scalar2=0.0,
                                op0=mybir.AluOpType.subtract,
                                op1=mybir.AluOpType.max)
                        else:
                            for mb in range(NB):
                                if mb < CLIP_ACT_N and "act" not in ablate:
                                    nc.scalar.activation(
                                        out=gt[:, mb, :], in_=psis[mb],
                                        func=AF.Relu, bias=mqn, scale=1.0)
                                else:
                                    nc.vector.tensor_scalar(
                                        out=gt[:, mb, :], in0=psis[mb],
                                        scalar1=mq, scalar2=0.0,
                                        op0=mybir.AluOpType.subtract,
                                        op1=mybir.AluOpType.max)
                        yt = ypool.tile([P, NB, W], f32, tag="y")
                        ydram = y[p].rearrange("(kb q) w -> q kb w", q=P)
                        halves = ((0, W // 2), (W // 2, W)) \
                            if (SEAM_SPLIT and p == NPLANES - 1) else ((0, W),)
                        first_half = True
                        for (lo, hi) in halves:
                            if "act" in ablate:
                                nc.scalar.copy(out=yt[:, :, lo:hi],
                                               in_=gt[:, :, lo:hi])
                            else:
                                nc.scalar.activation(out=yt[:, :, lo:hi],
                                                     in_=gt[:, :, lo:hi],
                                                     func=AF.Exp,
                                                     bias=0.0, scale=rsn)
                            if PE_KEEPALIVE and p == NPLANES - 1 \
                                    and first_half and "pe" not in ablate:
                                from concourse import mybir as _mb
                                keepalive(fin.bitcast(_mb.dt.float16)[:, 4:6])
                                keepalive(gt[:, NB - 1, 0:2])
                                keepalive(yt.bitcast(_mb.dt.float16)[
                                    :, NB - 1, lo:lo + 2])
                            nc.sync.dma_start(out=ydram[:, :, lo:hi],
                                              in_=yt[:, :, lo:hi])
                            first_half = False
                    else:
                        # per-mb: y = min(exp(-rs*q + mq*rs), 1) from PSUM
                        ydram = y[p].rearrange("(kb q) w -> q kb w", q=P)
                        for mb in range(NB):
                            ymb = ypool.tile([P, W], f32, tag="y",
                                             name=f"y{mb}")
                            if "act" in ablate:
                                nc.scalar.copy(out=ymb, in_=psis[mb])
                            else:
                                nc.scalar.activation(out=ymb, in_=psis[mb],
                                                     func=AF.Exp,
                                                     bias=bv, scale=rsn)
                            nc.vector.tensor_scalar_min(out=ymb, in0=ymb,
                                                        scalar1=1.0)
                            nc.sync.dma_start(out=ydram[:, mb, :], in_=ymb)

                # software-pipelined: pass 2 trails pass 1 by PIPE_DEPTH
                for p in range(NPLANES + PIPE_DEPTH):
                    if p < NPLANES:
                        front(p)
                    if p >= PIPE_DEPTH:
                        back(p - PIPE_DEPTH)

            if isinstance(reps, str) and reps.startswith("u"):
                for _ in range(int(reps[1:])):
                    emit_planes()
            elif reps == 1:
                emit_planes()
            else:
                from concourse import mybir as _mb
                u = UNROLL if reps % UNROLL == 0 else 1
                with tc.For_i(0, reps // u, 1,
                              hint_engines=(_mb.EngineType.PE,)):
                    for _ in range(u):
                        emit_planes()

    nc.compile()
    return nc


def get_program(reps=1):
    if reps not in _PROGRAM_CACHE:
        _PROGRAM_CACHE[reps] = build_program(reps)
    return _PROGRAM_CACHE[reps]


def build_v_matrices(k0, k1, k2):
    """fp16 banded Toeplitz matrices sqrt(w_s) * toeplitz(u_s) from the
    reference's 2-D depthwise kernels (u_s = column sums of the normalized
    2-D kernel, exact by separability)."""
    w = np.array([1.0, 0.75, 0.5], dtype=np.float64)
    w /= w.sum()
    out = []
    for s, k2d in enumerate((k0, k1, k2)):
        g = np.asarray(k2d)[0, 0].astype(np.float64)
        u = g.sum(axis=0)
        c = len(u) // 2
        V = np.zeros((H, W), dtype=np.float64)
        for d in range(-c, c + 1):
            V += np.diag(np.full(H - abs(d), u[c + d]), k=d)
        V *= np.sqrt(w[s])
        out.append(V.astype(np.float16))
    return out


def kernel(rgb_image, k0, k1, k2):
    from concourse.bass_utils import run_bass_kernel_spmd

    nc = get_program()
    v16 = build_v_matrices(k0, k1, k2)
    xs = np.ascontiguousarray(np.asarray(rgb_image, dtype=np.float32))
    B = xs.shape[0]
    per_core = B // N_CORES
    neg_ident = (-np.eye(P)).astype(np.float16)
    in_maps = []
    for c in range(N_CORES):
        m = {"x": xs[c * per_core:(c + 1) * per_core].reshape(NPLANES, H, W),
             "ident": neg_ident}
        for s in range(3):
            m[f"v{s}"] = v16[s]
        in_maps.append(m)
    res = run_bass_kernel_spmd(nc, in_maps, list(range(N_CORES)))
    out = np.empty((B, 3, H, W), dtype=np.float32)
    for c in range(N_CORES):
        out[c * per_core:(c + 1) * per_core] = (
            res.results[c]["y"].reshape(per_core, 3, H, W))
    return out


# revision 30
# speedup vs baseline: 1.1286x; 1.1286x over previous
"""ColorConstancy (multi-scale retinex) Trainium2 kernel.

Full-input contract: kernel(**inputs) takes the unsharded inputs from
setup_inputs() and returns the full (16, 3, 512, 512) float32 output.

Strategy (pure data parallel, batch sharded across 8 cores; 6 planes/core):
  log_img = ln(x + 1e-8)
  illum   = sum_s w_s * gauss2d_s(log_img)        (sigmas 2, 4, 8)
  refl    = log_img - illum
  out     = clip(exp((refl - mean) / (std_ddof1 + 1e-8)), 0, 1)

The 2-D Gaussian is separable: gauss2d_s(X) = U_s @ X @ U_s with U_s the
banded symmetric Toeplitz matrix of the 1-D kernel. Each pass is computed on
the TensorEngine as  pass(D) = D^T @ V  (lhsT = D blocks, rhs = V blocks), so
two passes give V^T X V = V X V with no explicit transposes. Folding
sqrt(w_s) into V_s makes illum a single PSUM accumulation in pass 2.

Key engine trick: pass 2's PSUM group also accumulates (-I)^T @ L, so PSUM
ends up holding  q = illum - log  = -refl  directly — the expensive DVE
tensor-tensor subtract (PSUM-source TT measured ~1.7us/tile) disappears.
Downstream works in q-space: bn_stats reads PSUM, clip(exp(z),0,1) =
exp(min(z,0)) becomes g = max(q, mean_q) (one DVE tensor-scalar from PSUM),
and y = exp(-rs*g + mean_q*rs) on ScalarE. PSUM evacuations are biased to
ACT (633ns/tile measured vs 1008 on DVE). Matmul operands are fp16
(full-rate PE, ~5e-4 rel precision).
"""

import numpy as np

N_CORES = 8
NPLANES = 6          # 2 batch images x 3 channels per core
H = W = 512
P = 128
NB = H // P          # 4 row blocks
CS = (6, 12, 24)     # band half-widths for sigma 2, 4, 8 (K = 13, 25, 49)
EPS = 1e-8
NPIX = H * W

_PROGRAM_CACHE = {}

# engine-balance knobs (tuned on HW):
#   EVAC_DVE_MOD: evacuations with (index % 12) < EVAC_DVE_MOD go to DVE
#   IDENT_TRICK: accumulate (-I)^T L into pass-2 PSUM (kills the DVE sub)
EVAC_DVE_MOD = 6
IDENT_TRICK = True
# LDW_SHARE: pass-1 kb-outer order + ldweights=False on the 2nd/3rd matmul
# sharing one stationary l16 block (saves ~1/3 of pass-1 LDWEIGHTS).
LDW_SHARE = False
# CLIP_ACT_N: how many of the 4 per-mb clip ops run on ACT as
# relu(q - mean_q) instead of on DVE as max(q - mean_q, 0); both produce
# r >= 0 and y = exp(-rs * r). Balances the clip load between engines.
CLIP_ACT_N = 0
# SEAM_SPLIT: halve the first plane's input DMA + Ln (along W) and the last
# plane's exp + output DMA, so the loop-seam critical path (barrier -> full
# DMA -> full Ln -> first matmul, and last exp -> full DMA -> barrier)
# pipelines at the halves. Steady-state planes stay whole-tile.
SEAM_SPLIT = True
# PIPE_DEPTH: software-pipeline lookahead in planes. 1 = pass 2 of plane p-1
# overlaps pass 1 of plane p; 2 = pass 2 trails by two planes, giving every
# engine a full extra plane of independent work (needs bufs=PIPE_DEPTH+1 on
# the front-side SBUF pools).
PIPE_DEPTH = 2
SBUF_BUFS = 2
# YDMA_ON_ACT: issue the output DMA from the ACT HWDGE queue (nc.scalar)
# instead of SP (nc.sync), splitting DMA issue/completion tracking across
# the two HWDGE engines.
YDMA_ON_ACT = False
PSA_BUFS = 3
PSI_BUFS = 4
# BACK_STYLE "max_exp": g = max(q, mq) on DVE (PSUM->SBUF), one whole-plane
#   exp on ACT, one whole-plane DMA out.
# BACK_STYLE "exp_min": per-mb exp straight from PSUM on ACT, per-mb clip-min
#   on DVE, per-mb DMA out (shorter serial tail, earlier PSUM release).
BACK_STYLE = "max_exp"
# QEVAC: copy q from PSUM to SBUF fp16 right away (engine "act" or "dve"),
# then bn_stats/max run on cheap SBUF fp16; psi pool shrinks to 2 and psA
# grows to 5. None = bn/max read PSUM directly (psi 4, psA 3).
QEVAC = None
# PE_KEEPALIVE: tiny dependency-chained matmuls through the first plane's
# input-DMA window and the last plane's stats/exp/DMA tail, so the PE never
# idles a full HAM activity window (idle > ~3.4us re-throttles the PE clock
# to 1.2 GHz; the For_i rep barrier otherwise exposes exactly such a gap).
PE_KEEPALIVE = True
# UNROLL: bodies per For_i iteration in the timing build (the For_i rep
# barrier + drain is a measurement-loop artifact; unrolling lets the
# software pipeline flow across plane-sets like the real one-shot kernel).
UNROLL = 2
# SIM_COMPAT: make pass-1 kb==0 matmuls write the full PSUM bank so CoreSim's
# chunk-granular pending-zero model accepts the banded accumulation (HW's
# per-element has_written handles the mixed case fine; sim-only, costs extra
# PE streaming so only enable for simulation analysis).
SIM_COMPAT = False


def _ncol(kb, c):
    """Output column range that input row block kb touches through a band-c kernel."""
    return max(0, P * kb - c), min(W, P * (kb + 1) + c)


def build_program(reps=1, ablate=()):
    """Build + compile the per-core Bass program. reps>1 wraps the whole
    computation in a hardware loop (for timing by subtraction).
    ablate: dev-only set of stage names to skip ("pe", "act", "evac") —
    output becomes wrong; used to attribute HW time."""
    ablate = set(ablate)
    import concourse.bacc as bacc
    import concourse.tile as tile
    from concourse import mybir, bass_isa

    f32 = mybir.dt.float32
    f16 = mybir.dt.float16
    AF = mybir.ActivationFunctionType

    # Steer Ln/Exp into the one table set containing both, so a single
    # ACT_TABLE_LOAD serves the whole kernel.
    from concourse.hw_specs import get_activation_tables
    _tabs = get_activation_tables("gen3")
    for _name, _fset in _tabs.items():
        if _name != "natural_log_exp_and_others":
            _fset.discard(AF.Ln)
            _fset.discard(AF.Exp)

    nc = bacc.Bacc("TRN2", target_bir_lowering=False, debug=False,
                   num_devices=N_CORES)
    x = nc.declare_dram_parameter("x", [NPLANES, H, W], f32, isOutput=False)
    vs = [nc.declare_dram_parameter(f"v{s}", [H, W], f16, isOutput=False)
          for s in range(3)]
    identd = nc.declare_dram_parameter("ident", [P, P], f16, isOutput=False)
    y = nc.declare_dram_parameter("y", [NPLANES, H, W], f32, isOutput=True)

    with tile.TileContext(nc) as tc:
        with (
            tc.tile_pool(name="consts", bufs=1) as consts,
            tc.tile_pool(name="xin", bufs=PIPE_DEPTH + 1) as xpool,
            tc.tile_pool(name="log16", bufs=PIPE_DEPTH + 1) as l16pool,
            tc.tile_pool(name="apool", bufs=PIPE_DEPTH + 1) as apool,
            tc.tile_pool(name="gpool", bufs=SBUF_BUFS) as gpool,
            tc.tile_pool(name="yout",
                         bufs=(6 if BACK_STYLE == "exp_min"
                               else SBUF_BUFS)) as ypool,
            tc.tile_pool(name="small", bufs=2) as spool,
            tc.tile_pool(name="psA", bufs=(5 if QEVAC else PSA_BUFS),
                         space="PSUM") as psA,
            tc.tile_pool(name="psI", bufs=(2 if QEVAC else PSI_BUFS),
                         space="PSUM") as psIp,
            tc.tile_pool(name="psS", bufs=1, space="PSUM") as psSp,
        ):
            # Banded blur matrices, resident for the whole kernel.
            # Layout [p, kb, n]: matrix row = kb*128 + p.
            V16 = []
            for s in range(3):
                vt = consts.tile([P, NB, W], f16, tag=f"v{s}")
                nc.sync.dma_start(
                    out=vt, in_=vs[s].rearrange("(kb p) n -> p kb n", p=P))
                V16.append(vt)
            epst = consts.tile([P, 1], f32, tag="eps")
            nc.vector.memset(epst, EPS)
            ones16 = consts.tile([P, P], f16, tag="ones16")
            nc.vector.memset(ones16, 1.0 / P)
            negI = consts.tile([P, P], f16, tag="negI")
            nc.sync.dma_start(out=negI, in_=identd[0:P])

            def emit_planes():
                state = {}

                def keepalive(rhs16):
                    # tiny matmul whose rhs depends on a tail/startup artifact
                    # — wakes the PE mid-gap so HAM never sees a full idle
                    # window. Output goes to the psS-tagged bank (rotated).
                    dps = psSp.tile([P, 2], f32, tag="psS", name="ka")
                    nc.tensor.matmul(dps[0:2, 0:2], ones16[:, 0:2], rhs16,
                                     start=True, stop=True)

                def front(p):
                    # load -> ln(fp16 out) -> pass 1 (A_s = L^T V_s)
                    xt = xpool.tile([P, NB, W], f32, tag="x")
                    xdram = x[p].rearrange("(kb q) w -> q kb w", q=P)
                    l16 = l16pool.tile([P, NB, W], f16, tag="l16")
                    halves = ((0, W // 2), (W // 2, W)) \
                        if (SEAM_SPLIT and p == 0) else ((0, W),)
                    for (lo, hi) in halves:
                        nc.sync.dma_start(out=xt[:, :, lo:hi],
                                          in_=xdram[:, :, lo:hi])
                        if PE_KEEPALIVE and p == 0 and lo == 0 \
                                and "pe" not in ablate:
                            from concourse import mybir as _mb
                            keepalive(xt.bitcast(_mb.dt.float16)[:, 0, 0:2])
                        if "act" in ablate:
                            nc.scalar.copy(out=l16[:, :, lo:hi],
                                           in_=xt[:, :, lo:hi])
                        else:
                            nc.scalar.activation(out=l16[:, :, lo:hi],
                                                 in_=xt[:, :, lo:hi],
                                                 func=AF.Ln,
                                                 bias=epst, scale=1.0)

                    A16 = [apool.tile([P, NB, W], f16, tag=f"a{s}", name=f"a16_{s}")
                           for s in range(3)]
                    if "evac" in ablate:
                        for s in range(3):
                            nc.vector.memset(A16[s], 0.5)
                    nevac = 0
                    if "pe" in ablate:
                        dummy = l16pool.tile([P, W], f32, tag="dummy",
                                             name="dummy")
                        nc.vector.memset(dummy, 0.25)
                    for mb in range(NB):
                        if "pe" in ablate:
                            ps = [dummy, dummy, dummy]
                        else:
                            ps = [psA.tile([P, W], f32, tag="ps", name=f"ps{i}")
                                  for i in range(3)]
                            if LDW_SHARE:
                                # kb outer: the 3 sigma matmuls of one kb
                                # share the stationary l16 block; skip the
                                # redundant LDWEIGHTS on the 2nd and 3rd.
                                for kb in range(NB):
                                    for s in range(3):
                                        lo, hi = _ncol(kb, CS[s])
                                        if SIM_COMPAT and kb == 0:
                                            lo, hi = 0, W
                                        inst = nc.tensor.matmul(
                                            ps[s][:, lo:hi],
                                            l16[:, kb, P * mb:P * (mb + 1)],
                                            V16[s][:, kb, lo:hi],
                                            start=(kb == 0),
                                            stop=(kb == NB - 1),
                                        )
                                        if s > 0:
                                            inst.ldweights = False
                            else:
                                for s in range(3):
                                    for kb in range(NB):
                                        lo, hi = _ncol(kb, CS[s])
                                        if SIM_COMPAT and kb == 0:
                                            lo, hi = 0, W
                                        nc.tensor.matmul(
                                            ps[s][:, lo:hi],
                                            l16[:, kb, P * mb:P * (mb + 1)],
                                            V16[s][:, kb, lo:hi],
                                            start=(kb == 0),
                                            stop=(kb == NB - 1),
                                        )
                        for s in range(3):
                            # evacuate PSUM -> SBUF fp16, split DVE/ACT
                            if "evac" in ablate:
                                continue
                            if nevac % 12 >= EVAC_DVE_MOD:
                                nc.scalar.copy(out=A16[s][:, mb, :], in_=ps[s])
                            else:
                                nc.vector.tensor_copy(out=A16[s][:, mb, :],
                                                      in_=ps[s])
                            nevac += 1
                    state[p] = (l16, A16)

                def back(p):
                    # pass 2: q = illum - log accumulated in PSUM
                    # (banded A_s^T V_s matmuls + (-I)^T L), then stats and
                    # y = exp(min(refl - mean, 0)/std) all in q-space.
                    l16, A16 = state.pop(p)
                    st6 = spool.tile([P, NB, 6], f32, tag="st6")
                    psis = []
                    q16 = (gpool.tile([P, NB, W], f16, tag="q16", name="q16")
                           if QEVAC else None)
                    for mb in range(NB):
                        if "pe" in ablate:
                            psi = l16[:, mb, :]
                        else:
                            psi = psIp.tile([P, W], f32, tag="psi",
                                            name=f"psi{mb}")
                            nc.tensor.matmul(psi, negI, l16[:, mb, :],
                                             start=True, stop=False)
                            for s in range(3):
                                for kb in range(NB):
                                    lo, hi = _ncol(kb, CS[s])
                                    nc.tensor.matmul(
                                        psi[:, lo:hi],
                                        A16[s][:, kb, P * mb:P * (mb + 1)],
                                        V16[s][:, kb, lo:hi],
                                        start=False,
                                        stop=(s == 2 and kb == NB - 1),
                                    )
                        if QEVAC == "act":
                            nc.scalar.copy(out=q16[:, mb, :], in_=psi)
                        elif QEVAC == "dve":
                            nc.vector.tensor_copy(out=q16[:, mb, :], in_=psi)
                        if QEVAC:
                            nc.vector.bn_stats(out=st6[:, mb, :],
                                               in_=q16[:, mb, :])
                        else:
                            nc.vector.bn_stats(out=st6[:, mb, :], in_=psi)
                            psis.append(psi)

                    # plane-wide mean/var of q: per-partition bn stats, then a
                    # ones-weights matmul sums [mean_p, E[x^2]_p] across
                    # partitions AND broadcasts to all partitions.
                    mv = spool.tile([P, 2], f32, tag="mv")
                    nc.vector.bn_aggr(out=mv, in_=st6)
                    t2 = spool.tile([P, 2], f16, tag="t2")
                    nc.vector.tensor_mul(out=t2[:, 1:2], in0=mv[:, 0:1],
                                         in1=mv[:, 0:1])
                    nc.vector.tensor_add(out=t2[:, 1:2], in0=t2[:, 1:2],
                                         in1=mv[:, 1:2])
                    nc.vector.tensor_copy(out=t2[:, 0:1], in_=mv[:, 0:1])
                    psS = psSp.tile([P, 2], f32, tag="psS")
                    nc.tensor.matmul(psS, ones16, t2, start=True, stop=True)

                    fin = spool.tile([P, 6], f32, tag="fin")
                    mq = fin[:, 0:1]     # mean of q  (= -mean(refl))
                    tmp = fin[:, 1:2]    # var -> rs
                    rs = fin[:, 2:3]
                    rsn = fin[:, 3:4]    # -rs
                    mqn = fin[:, 4:5]    # -mq (ACT relu bias)
                    nc.vector.tensor_copy(out=mq, in_=psS[:, 0:1])
                    sq = spool.tile([P, 1], f32, tag="sq")
                    nc.vector.tensor_mul(out=sq, in0=mq, in1=mq)
                    nc.vector.tensor_sub(out=tmp, in0=psS[:, 1:2], in1=sq)
                    # rs = 1/std = exp(-0.5*ln(var * N/(N-1)))  (ddof=1; the
                    # +eps on std is 1e-8 vs std~O(1), folded away). Ln/Exp
                    # share one ACT table set.
                    nc.scalar.activation(out=tmp, in_=tmp, func=AF.Ln,
                                         scale=float(NPIX) / (NPIX - 1))
                    nc.scalar.activation(out=rs, in_=tmp, func=AF.Exp,
                                         scale=-0.5)
                    nc.vector.tensor_scalar_mul(out=rsn, in0=rs, scalar1=-1.0)
                    if CLIP_ACT_N > 0:
                        nc.vector.tensor_scalar_mul(out=mqn, in0=mq,
                                                    scalar1=-1.0)

                    if BACK_STYLE == "max_exp":
                        # r = max(q - mq, 0); then y = exp(-rs*r)
                        gt = gpool.tile([P, NB, W], f16, tag="g")
                        if QEVAC:
                            nc.vector.tensor_scalar(
                                out=gt, in0=q16, scalar1=mq, scalar2=0.0,
                                op0=mybir.AluOpType.subtract,
                                op1=mybir.AluOpType.max)
                        else:
                            for mb in range(NB):
                                if mb < CLIP_ACT_N and "act" not in ablate:
                                    nc.scalar.activation(
                                        out=gt[:, mb, :], in_=psis[mb],
                                        func=AF.Relu, bias=mqn, scale=1.0)
                                else:
                                    nc.vector.tensor_scalar(
                                        out=gt[:, mb, :], in0=psis[mb],
                                        scalar1=mq, scalar2=0.0,
                                        op0=mybir.AluOpType.subtract,
                                        op1=mybir.AluOpType.max)
                        yt = ypool.tile([P, NB, W], f32, tag="y")
                        ydram = y[p].rearrange("(kb q) w -> q kb w", q=P)
                        halves = ((0, W // 2), (W // 2, W)) \
                            if (SEAM_SPLIT and p == NPLANES - 1) else ((0, W),)
                        first_half = True
                        for (lo, hi) in halves:
                            if "act" in ablate:
                                nc.scalar.copy(out=yt[:, :, lo:hi],
                                               in_=gt[:, :, lo:hi])
                            else:
                                nc.scalar.activation(out=yt[:, :, lo:hi],
                                                     in_=gt[:, :, lo:hi],
                                                     func=AF.Exp,
                                                     bias=0.0, scale=rsn)
                            if PE_KEEPALIVE and p == NPLANES - 1 \
                                    and first_half and "pe" not in ablate:
                                from concourse import mybir as _mb
                                keepalive(fin.bitcast(_mb.dt.float16)[:, 4:6])
                                keepalive(gt[:, NB - 1, 0:2])
                                keepalive(yt.bitcast(_mb.dt.float16)[
                                    :, NB - 1, lo:lo + 2])
                            if YDMA_ON_ACT:
                                nc.scalar.dma_start(out=ydram[:, :, lo:hi],
                                                    in_=yt[:, :, lo:hi])
                            else:
                                nc.sync.dma_start(out=ydram[:, :, lo:hi],
                                                  in_=yt[:, :, lo:hi])
                            first_half = False
                    else:
                        # per-mb: y = min(exp(-rs*q + mq*rs), 1) from PSUM
                        ydram = y[p].rearrange("(kb q) w -> q kb w", q=P)
                        for mb in range(NB):
                            ymb = ypool.tile([P, W], f32, tag="y",
                                             name=f"y{mb}")
                            if "act" in ablate:
                                nc.scalar.copy(out=ymb, in_=psis[mb])
                            else:
                                nc.scalar.activation(out=ymb, in_=psis[mb],
                                                     func=AF.Exp,
                                                     bias=bv, scale=rsn)
                            nc.vector.tensor_scalar_min(out=ymb, in0=ymb,
                                                        scalar1=1.0)
                            nc.sync.dma_start(out=ydram[:, mb, :], in_=ymb)

                # software-pipelined: pass 2 trails pass 1 by PIPE_DEPTH
                for p in range(NPLANES + PIPE_DEPTH):
                    if p < NPLANES:
                        front(p)
                    if p >= PIPE_DEPTH:
                        back(p - PIPE_DEPTH)

            if isinstance(reps, str) and reps.startswith("u"):
                for _ in range(int(reps[1:])):
                    emit_planes()
            elif reps == 1:
                emit_planes()
            else:
                from concourse import mybir as _mb
                u = UNROLL if reps % UNROLL == 0 else 1
                with tc.For_i(0, reps // u, 1,
                              hint_engines=(_mb.EngineType.PE,)):
                    for _ in range(u):
                        emit_planes()

    nc.compile()
    return nc


def get_program(reps=1):
    if reps not in _PROGRAM_CACHE:
        _PROGRAM_CACHE[reps] = build_program(reps)
    return _PROGRAM_CACHE[reps]


def build_v_matrices(k0, k1, k2):
    """fp16 banded Toeplitz matrices sqrt(w_s) * toeplitz(u_s) from the
    reference's 2-D depthwise kernels (u_s = column sums of the normalized
    2-D kernel, exact by separability)."""
    w = np.array([1.0, 0.75, 0.5], dtype=np.float64)
    w /= w.sum()
    out = []
    for s, k2d in enumerate((k0, k1, k2)):
        g = np.asarray(k2d)[0, 0].astype(np.float64)
        u = g.sum(axis=0)
        c = len(u) // 2
        V = np.zeros((H, W), dtype=np.float64)
        for d in range(-c, c + 1):
            V += np.diag(np.full(H - abs(d), u[c + d]), k=d)
        V *= np.sqrt(w[s])
        out.append(V.astype(np.float16))
    return out


def kernel(rgb_image, k0, k1, k2):
    from concourse.bass_utils import run_bass_kernel_spmd

    nc = get_program()
    v16 = build_v_matrices(k0, k1, k2)
    xs = np.ascontiguousarray(np.asarray(rgb_image, dtype=np.float32))
    B = xs.shape[0]
    per_core = B // N_CORES
    neg_ident = (-np.eye(P)).astype(np.float16)
    in_maps = []
    for c in range(N_CORES):
        m = {"x": xs[c * per_core:(c + 1) * per_core].reshape(NPLANES, H, W),
             "ident": neg_ident}
        for s in range(3):
            m[f"v{s}"] = v16[s]
        in_maps.append(m)
    res = run_bass_kernel_spmd(nc, in_maps, list(range(N_CORES)))
    out = np.empty((B, 3, H, W), dtype=np.float32)
    for c in range(N_CORES):
        out[c * per_core:(c + 1) * per_core] = (
            res.results[c]["y"].reshape(per_core, 3, H, W))
    return out
